# revision 1
# baseline (speedup 1.0000x reference)
"""Llama4-style MoE experts (grouped SwiGLU MLP) on Trainium2, 8 NeuronCores.

Expert-parallel: core i runs expert i's full MLP on its 1024-token slice:
    out = (up * silu(gate)) @ W2,  [gate|up] = h @ W1
Per-core shapes: h [1024, 2048], W1 [2048, 8192], W2 [4096, 2048].

Matmuls run in bf16 on the TensorEngine (1 cycle/row); f32 inputs are
cast on the VectorEngine, which is otherwise mostly idle. h is
transposed on-chip via PE transpose-mode; the first f-block of W1 is
then consumed in narrow half-blocks whose stripes arrive in exactly the
order the PE consumes them, so matmul-1 paces off the DMA stream.
"""

from contextlib import ExitStack

import numpy as np

import concourse.bass as bass
import concourse.mybir as mybir
import concourse.tile as tile
from concourse import bacc
from concourse.bass_utils import run_bass_kernel_spmd
from concourse.masks import make_identity

N_CORES = 8
P = 128
TB = 512  # moving-operand free-dim block (one PSUM bank of f32)

F32 = mybir.dt.float32
BF16 = mybir.dt.bfloat16
ACT_SIGMOID = mybir.ActivationFunctionType.Sigmoid

# Per-core problem dims (full problem: 8 experts x 1024 tokens, H=2048, F=4096)
T = 1024
H = 2048
F = 4096


def build_kernel_body(tc, T=T, H=H, F=F):
    nc = tc.nc
    h_d = nc.dram_tensor("hidden_states", [T, H], F32, kind="ExternalInput").ap()
    w1_d = nc.dram_tensor("gate_up_proj", [H, 2 * F], F32, kind="ExternalInput").ap()
    w2_d = nc.dram_tensor("down_proj", [F, H], F32, kind="ExternalInput").ap()
    out_d = nc.dram_tensor("out", [T, H], F32, kind="ExternalOutput").ap()

    n_ht = H // P          # h-tiles (contraction tiles of matmul 1)
    n_ft = F // P          # f-tiles (rows of act; contraction tiles of matmul 2)
    n_tt = T // P          # token tiles (psum partition tiles of matmul 2)
    n_tb = T // TB         # token free-dim blocks in matmul 1
    n_fb = F // TB         # 512-wide f blocks of W1 (per gate/up half)
    n_hb = H // TB         # 512-wide h blocks of W2

    with ExitStack() as ctx:
        const = ctx.enter_context(tc.tile_pool(name="const", bufs=1))
        hcolp = ctx.enter_context(tc.tile_pool(name="hcolp", bufs=6))
        htp = ctx.enter_context(tc.tile_pool(name="htp", bufs=n_ht))
        actp = ctx.enter_context(tc.tile_pool(name="actp", bufs=n_ft))
        wf = ctx.enter_context(tc.tile_pool(name="wf", bufs=6))
        w1b = ctx.enter_context(tc.tile_pool(name="w1b", bufs=80))
        w2b = ctx.enter_context(tc.tile_pool(name="w2b", bufs=n_ft + 10))
        silp = ctx.enter_context(tc.tile_pool(name="silp", bufs=4))
        outp = ctx.enter_context(tc.tile_pool(name="outp", bufs=3))
        ps = ctx.enter_context(tc.tile_pool(name="ps", bufs=8, space="PSUM"))

        ident = const.tile([P, P], F32, tag="ident", name="ident")
        make_identity(nc, ident)

        ht = [htp.tile([P, T], BF16, tag="ht", name=f"ht{i}") for i in range(n_ht)]
        act = [actp.tile([P, T], BF16, tag="act", name=f"act{i}") for i in range(n_ft)]

        def load_w1_block(fb):
            """DMA + cast one 512-wide f-block of W1 (gate + up halves).
            Returns wg[hh][i], wu[hh][i] bf16 [128,128] tiles."""
            wg = [[None] * (TB // P) for _ in range(n_ht)]
            wu = [[None] * (TB // P) for _ in range(n_ht)]
            for hh in range(n_ht):
                _load_w1_stripe(fb, hh, wg, wu)
            return wg, wu

        def _load_w1_stripe(fb, hh, wg, wu, i0=0, ni=None, tag="wf", bufs=None):
            """DMA one [128, ni*128] slice of W1 (gate+up) and cast to bf16
            [128,128] tiles wg/wu[hh][i0..i0+ni-1]."""
            if ni is None:
                ni = TB // P
            w = ni * P
            c0 = fb * TB + i0 * P
            kw = {} if bufs is None else {"bufs": bufs}
            sg = wf.tile([P, w], F32, tag=tag, name=f"w1g_{fb}_{hh}_{i0}", **kw)
            nc.sync.dma_start(sg[:], w1_d[hh * P : (hh + 1) * P, c0 : c0 + w])
            su = wf.tile([P, w], F32, tag=tag, name=f"w1u_{fb}_{hh}_{i0}", **kw)
            nc.sync.dma_start(
                su[:], w1_d[hh * P : (hh + 1) * P, F + c0 : F + c0 + w]
            )
            for k in range(ni):
                i = i0 + k
                gb = w1b.tile([P, P], BF16, tag="w1b", name=f"w1gb_{fb}_{hh}_{i}")
                nc.vector.tensor_copy(out=gb[:], in_=sg[:, k * P : (k + 1) * P])
                ub = w1b.tile([P, P], BF16, tag="w1b", name=f"w1ub_{fb}_{hh}_{i}")
                nc.vector.tensor_copy(out=ub[:], in_=su[:, k * P : (k + 1) * P])
                wg[hh][i] = gb
                wu[hh][i] = ub

        def swiglu(fi, tb, pg, pu):
            sig = silp.tile([P, TB], BF16, tag="silp", name=f"sig{fi}_{tb}")
            nc.scalar.activation(sig[:], pg[:], ACT_SIGMOID)
            tmp = silp.tile([P, TB], BF16, tag="tmpp", name=f"tmp{fi}_{tb}")
            nc.vector.tensor_mul(out=tmp[:], in0=pu[:], in1=sig[:])
            nc.vector.tensor_mul(
                out=act[fi][:, tb * TB : (tb + 1) * TB], in0=tmp[:], in1=pg[:]
            )

        def mm1_block(fb, wg, wu, i_range=None, h_outer=False):
            """Matmul-1 + SwiGLU for (part of) one 512-wide f-block.

            h_outer orders matmuls h-major with all psum groups live so the
            PE can consume W1 stripes as they arrive (startup pipelining).
            The stationary tile is reused across both token halves either
            way, halving LDWEIGHTS traffic.
            """
            if i_range is None:
                i_range = range(TB // P)
            pgs, pus = {}, {}

            def alloc(i):
                fi = fb * (TB // P) + i
                pgs[i] = [
                    ps.tile([P, TB], F32, tag="ps", name=f"pg{fi}_{tb}")
                    for tb in range(n_tb)
                ]
                pus[i] = [
                    ps.tile([P, TB], F32, tag="ps", name=f"pu{fi}_{tb}")
                    for tb in range(n_tb)
                ]

            def mm(i, hh):
                first, last = hh == 0, hh == n_ht - 1
                for p, w in ((pgs[i], wg), (pus[i], wu)):
                    for tb in range(n_tb):
                        nc.tensor.matmul(
                            p[tb][:],
                            lhsT=w[hh][i][:],
                            rhs=ht[hh][:, tb * TB : (tb + 1) * TB],
                            start=first,
                            stop=last,
                        )

            if h_outer:
                for i in i_range:
                    alloc(i)
                for hh in range(n_ht):
                    for i in i_range:
                        mm(i, hh)
            else:
                for i in i_range:
                    alloc(i)
                    for hh in range(n_ht):
                        mm(i, hh)
                    for tb in range(n_tb):
                        swiglu(fb * (TB // P) + i, tb, pgs[i][tb], pus[i][tb])
                return
            for i in i_range:
                for tb in range(n_tb):
                    swiglu(fb * (TB // P) + i, tb, pgs[i][tb], pus[i][tb])

        # ---- Phase A: transpose h -> hT bf16 ----
        for ti in range(n_tt):
            hr = hcolp.tile([P, H], F32, tag="hrow", name=f"hrow{ti}", bufs=2)
            nc.sync.dma_start(hr[:], h_d[ti * P : (ti + 1) * P, :])
            for hh in range(n_ht):
                pt = ps.tile([P, TB], F32, tag="ps", name=f"tp{ti}_{hh}")
                nc.tensor.transpose(pt[:, :P], hr[:, hh * P : (hh + 1) * P], ident)
                nc.vector.tensor_copy(
                    out=ht[hh][:, ti * P : (ti + 1) * P], in_=pt[:, :P]
                )

        # ---- Phase B: G = h @ W1, act = up * silu(gate), stored [f, t] bf16 ----
        # fb=0 runs in narrow half-blocks whose W1 stripes arrive in exactly
        # the order the PE consumes them, so matmuls pace off the DMA stream
        # right after the transposes instead of waiting for a full f-block.
        wg0 = [[None] * (TB // P) for _ in range(n_ht)]
        wu0 = [[None] * (TB // P) for _ in range(n_ht)]
        half = max(1, (TB // P) // 2)
        for hh in range(n_ht):
            _load_w1_stripe(0, hh, wg0, wu0, i0=0, ni=half, tag="wfh")
        mm1_block(0, wg0, wu0, i_range=range(0, half))
        if half < TB // P:
            for hh in range(n_ht):
                _load_w1_stripe(
                    0, hh, wg0, wu0, i0=half, ni=(TB // P) - half, tag="wfh"
                )
            mm1_block(0, wg0, wu0, i_range=range(half, TB // P))
        for fb in range(1, n_fb):
            wg, wu = load_w1_block(fb)
            mm1_block(fb, wg, wu)

        # ---- Phase C: out = act @ W2, contracting over f ----
        for hb in range(n_hb):
            b2 = []
            for f in range(n_ft):
                s2 = wf.tile([P, TB], F32, tag="wf", name=f"w2f_{hb}_{f}")
                nc.sync.dma_start(
                    s2[:], w2_d[f * P : (f + 1) * P, hb * TB : (hb + 1) * TB]
                )
                t2 = w2b.tile([P, TB], BF16, tag="w2b", name=f"w2b_{hb}_{f}")
                nc.vector.tensor_copy(out=t2[:], in_=s2[:])
                b2.append(t2)
            for tt in range(n_tt):
                po = ps.tile([P, TB], F32, tag="ps", name=f"po{hb}_{tt}")
                for f in range(n_ft):
                    nc.tensor.matmul(
                        po[:],
                        lhsT=act[f][:, tt * P : (tt + 1) * P],
                        rhs=b2[f][:],
                        start=(f == 0),
                        stop=(f == n_ft - 1),
                    )
                ob = outp.tile([P, TB], F32, tag="outp", name=f"ob{hb}_{tt}")
                nc.vector.tensor_copy(out=ob[:], in_=po[:])
                nc.sync.dma_start(
                    out_d[tt * P : (tt + 1) * P, hb * TB : (hb + 1) * TB], ob[:]
                )


def build_nc(T=T, H=H, F=F):
    nc = bacc.Bacc(
        "TRN2", target_bir_lowering=False, debug=False, enable_asserts=False
    )
    with tile.TileContext(nc) as tc:
        build_kernel_body(tc, T=T, H=H, F=F)
    nc.compile()
    return nc


_NC_CACHE = None


def run(hidden_states, gate_up_proj, down_proj, trace=False, **kw):
    """Run on the 8 NeuronCores; returns (output, BassKernelResults)."""
    global _NC_CACHE
    if _NC_CACHE is None:
        _NC_CACHE = build_nc()
    nc = _NC_CACHE

    hs = np.ascontiguousarray(np.asarray(hidden_states), dtype=np.float32)
    gup = np.ascontiguousarray(np.asarray(gate_up_proj), dtype=np.float32)
    dp = np.ascontiguousarray(np.asarray(down_proj), dtype=np.float32)
    assert hs.shape == (N_CORES * T, H), hs.shape
    assert gup.shape == (N_CORES, H, 2 * F), gup.shape
    assert dp.shape == (N_CORES, F, H), dp.shape

    in_maps = [
        {
            "hidden_states": np.ascontiguousarray(hs[i * T : (i + 1) * T]),
            "gate_up_proj": np.ascontiguousarray(gup[i]),
            "down_proj": np.ascontiguousarray(dp[i]),
        }
        for i in range(N_CORES)
    ]
    res = run_bass_kernel_spmd(
        nc, in_maps, core_ids=list(range(N_CORES)), trace=trace, **kw
    )
    out = np.concatenate(
        [res.results[i]["out"] for i in range(N_CORES)], axis=0
    ).astype(np.float32)
    return out, res


def kernel(hidden_states, gate_up_proj, down_proj):
    out, _ = run(hidden_states, gate_up_proj, down_proj, trace=False)
    return out



# revision 6
# speedup vs baseline: 1.0554x; 1.0554x over previous
"""Llama4-style MoE experts (grouped SwiGLU MLP) on Trainium2, 8 NeuronCores.

Expert-parallel: core i runs expert i's full MLP on its 1024-token slice:
    out = (up * silu(gate)) @ W2,  [gate|up] = h @ W1
Per-core shapes: hT [2048, 1024] (host pre-transposed), W1 [2048, 8192],
W2 [4096, 2048].

hidden_states is transposed on the host so h arrives contraction-major;
no on-chip transposes. All matmuls run bf16 on the TensorEngine (f32
operands are cast on the otherwise-idle VectorEngine). Phase B (h @ W1 +
SwiGLU) paces MM-by-MM off the interleaved h/W1 DMA stream at startup.
Phase C (act @ W2) keeps all 8 token-tile PSUM accumulators live and
walks the f contraction innermost, so each W2 tile is consumed in one
burst right after it lands and a small rotating pool gives a deep
prefetch; the last 4 f-steps are issued per-token-tile so output drains
stagger instead of bunching at the end.
"""

from contextlib import ExitStack

import numpy as np

import concourse.bass as bass
import concourse.mybir as mybir
import concourse.tile as tile
from concourse import bacc
from concourse.bass_utils import run_bass_kernel_spmd

N_CORES = 8
P = 128
TB = 512  # PSUM free-dim block (one f32 bank)

F32 = mybir.dt.float32
BF16 = mybir.dt.bfloat16
ACT_SIGMOID = mybir.ActivationFunctionType.Sigmoid

# Per-core problem dims (full problem: 8 experts x 1024 tokens, H=2048, F=4096)
T = 1024
H = 2048
F = 4096


def build_kernel_body(tc, T=T, H=H, F=F):
    nc = tc.nc
    h_d = nc.dram_tensor("hidden_states", [H, T], F32, kind="ExternalInput").ap()
    w1_d = nc.dram_tensor("gate_up_proj", [H, 2 * F], F32, kind="ExternalInput").ap()
    w2_d = nc.dram_tensor("down_proj", [F, H], F32, kind="ExternalInput").ap()
    out_d = nc.dram_tensor("out", [T, H], F32, kind="ExternalOutput").ap()

    n_ht = H // P   # 16 contraction tiles of matmul 1
    n_fb = F // TB  # 8  512-wide f blocks of W1 (per gate/up half)
    n_if = TB // P  # 4  f-tiles per block
    n_tb = T // TB  # 2  token free-dim blocks in matmul 1
    n_ft = F // P   # 32 f-tiles (contraction tiles of matmul 2)
    n_tt = T // P   # 8  token psum tiles of matmul 2
    n_hb = H // TB  # 4  512-wide h blocks of W2
    C_TAIL = 4      # f-steps of matmul 2 issued per-tt to stagger drains

    with ExitStack() as ctx:
        hstage = ctx.enter_context(tc.tile_pool(name="hstage", bufs=2))
        wstage = ctx.enter_context(tc.tile_pool(name="wstage", bufs=8))
        htp = ctx.enter_context(tc.tile_pool(name="htp", bufs=n_ht))
        actp = ctx.enter_context(tc.tile_pool(name="actp", bufs=n_ft))
        w1bp = ctx.enter_context(tc.tile_pool(name="w1bp", bufs=44))
        w2bp = ctx.enter_context(tc.tile_pool(name="w2bp", bufs=22))
        silp = ctx.enter_context(tc.tile_pool(name="silp", bufs=4))
        outp = ctx.enter_context(tc.tile_pool(name="outp", bufs=3))
        ps = ctx.enter_context(tc.tile_pool(name="ps", bufs=8, space="PSUM"))

        ht = [htp.tile([P, T], BF16, tag="ht", name=f"ht{i}") for i in range(n_ht)]
        act = [actp.tile([P, T], BF16, tag="act", name=f"act{i}") for i in range(n_ft)]

        def load_h(hh):
            st = hstage.tile([P, T], F32, tag="hst", name=f"hst{hh}")
            nc.sync.dma_start(st[:], h_d[hh * P : (hh + 1) * P, :])
            nc.vector.tensor_copy(out=ht[hh][:], in_=st[:])

        def load_w1(fb, x, hh):
            """DMA + cast one [128, 512] stripe of W1 half x (0=gate, 1=up)."""
            c0 = x * F + fb * TB
            st = wstage.tile([P, TB], F32, tag="wst", name=f"w1s_{fb}_{x}_{hh}")
            nc.sync.dma_start(st[:], w1_d[hh * P : (hh + 1) * P, c0 : c0 + TB])
            wb = w1bp.tile([P, TB], BF16, tag="w1b", name=f"w1b_{fb}_{x}_{hh}")
            nc.vector.tensor_copy(out=wb[:], in_=st[:])
            return wb

        w2t = {}  # (hb, f) -> bf16 tile

        def load_w2(hb, f):
            """DMA + cast one [128, 512] tile of W2 (f-tile f, h-block hb)."""
            st = wstage.tile([P, TB], F32, tag="wst", name=f"w2s_{hb}_{f}")
            nc.sync.dma_start(
                st[:], w2_d[f * P : (f + 1) * P, hb * TB : (hb + 1) * TB]
            )
            wb = w2bp.tile([P, TB], BF16, tag="w2b", name=f"w2b_{hb}_{f}")
            nc.vector.tensor_copy(out=wb[:], in_=st[:])
            w2t[(hb, f)] = wb

        def swiglu(fi, pg, pu):
            for tb in range(n_tb):
                sig = silp.tile([P, TB], BF16, tag="silp", name=f"sig{fi}_{tb}")
                nc.scalar.activation(sig[:], pg[tb][:], ACT_SIGMOID)
                tmp = silp.tile([P, TB], BF16, tag="tmpp", name=f"tmp{fi}_{tb}")
                nc.vector.tensor_mul(out=tmp[:], in0=pu[tb][:], in1=sig[:])
                nc.vector.tensor_mul(
                    out=act[fi][:, tb * TB : (tb + 1) * TB],
                    in0=tmp[:],
                    in1=pg[tb][:],
                )

        # ---- Phase A/B startup: interleave h stripes with W1-fb0 stripes so
        # the first matmuls start as soon as ht[0] + the first stripe pair
        # land, then pace MM-by-MM off the DMA stream.
        w1t = {0: {0: [None] * n_ht, 1: [None] * n_ht}}
        for r in range(4):
            for hh in range(4 * r, 4 * r + 4):
                load_h(hh)
            for hh in range(4 * r, 4 * r + 4):
                for x in range(2):
                    w1t[0][x][hh] = load_w1(0, x, hh)

        def mm1_block(fb, prefetch):
            """Matmul-1 + SwiGLU for one 512-wide f-block (gate+up halves).

            prefetch: list of thunks; one is popped and run after each
            i-phase's matmuls to place next-block DMAs in program order.
            """
            wg, wu = w1t[fb][0], w1t[fb][1]
            prev = None
            for i in range(n_if):
                fi = fb * n_if + i
                pg = [
                    ps.tile([P, TB], F32, tag="ps", name=f"pg{fi}_{tb}")
                    for tb in range(n_tb)
                ]
                pu = [
                    ps.tile([P, TB], F32, tag="ps", name=f"pu{fi}_{tb}")
                    for tb in range(n_tb)
                ]
                for hh in range(n_ht):
                    first, last = hh == 0, hh == n_ht - 1
                    for p, w in ((pg, wg), (pu, wu)):
                        lw = w[hh][:, i * P : (i + 1) * P]
                        for tb in range(n_tb):
                            nc.tensor.matmul(
                                p[tb][:],
                                lhsT=lw,
                                rhs=ht[hh][:, tb * TB : (tb + 1) * TB],
                                start=first,
                                stop=last,
                            )
                if prefetch:
                    prefetch.pop(0)()
                if prev is not None:
                    swiglu(*prev)
                prev = (fi, pg, pu)
            swiglu(*prev)

        def w1_prefetch(fb):
            """Thunks loading next f-block's stripes, 8 per i-phase."""
            w1t[fb] = {0: [None] * n_ht, 1: [None] * n_ht}

            def mk(h0):
                def go():
                    for hh in range(h0, h0 + 4):
                        for x in range(2):
                            w1t[fb][x][hh] = load_w1(fb, x, hh)

                return go

            return [mk(4 * r) for r in range(4)]

        def w2_prefetch(hb, f0, n):
            def go():
                for f in range(f0, f0 + n):
                    load_w2(hb, f)

            return go

        for fb in range(n_fb):
            if fb < n_fb - 1:
                pf = w1_prefetch(fb + 1)
            elif fb == n_fb - 1:
                # last W1 block: prefetch W2 h-block 0 second half
                pf = [w2_prefetch(0, 16 + 4 * r, 4) for r in range(4)]
            if fb == n_fb - 2:
                # second-to-last: interleave W2 h-block 0 first half
                pf = [
                    (lambda a, b: lambda: (a(), b()))(pf[r], w2_prefetch(0, 4 * r, 4))
                    for r in range(4)
                ]
            mm1_block(fb, pf)

        # ---- Phase C: out = act @ W2, f contraction innermost across all 8
        # token-tile accumulators; W2 tiles stream one-hb-ahead through the
        # rotating pool.
        for hb in range(n_hb):
            po = [
                ps.tile([P, TB], F32, tag="ps", name=f"po{hb}_{tt}")
                for tt in range(n_tt)
            ]
            for f in range(n_ft):
                if hb + 1 < n_hb:
                    load_w2(hb + 1, f)
                if f < n_ft - C_TAIL:
                    w = w2t.pop((hb, f))
                    for tt in range(n_tt):
                        nc.tensor.matmul(
                            po[tt][:],
                            lhsT=act[f][:, tt * P : (tt + 1) * P],
                            rhs=w[:],
                            start=(f == 0),
                            stop=False,
                        )
            # staggered tail: per token tile, finish the last f-steps, then
            # drain + store while the next token tile computes
            for tt in range(n_tt):
                for f in range(n_ft - C_TAIL, n_ft):
                    nc.tensor.matmul(
                        po[tt][:],
                        lhsT=act[f][:, tt * P : (tt + 1) * P],
                        rhs=w2t[(hb, f)][:],
                        start=False,
                        stop=(f == n_ft - 1),
                    )
                ob = outp.tile([P, TB], F32, tag="outp", name=f"ob{hb}_{tt}")
                nc.vector.tensor_copy(out=ob[:], in_=po[tt][:])
                nc.sync.dma_start(
                    out_d[tt * P : (tt + 1) * P, hb * TB : (hb + 1) * TB], ob[:]
                )
            for f in range(n_ft - C_TAIL, n_ft):
                del w2t[(hb, f)]


def build_nc(T=T, H=H, F=F):
    nc = bacc.Bacc(
        "TRN2", target_bir_lowering=False, debug=False, enable_asserts=False
    )
    with tile.TileContext(nc) as tc:
        build_kernel_body(tc, T=T, H=H, F=F)
    nc.compile()
    return nc


_NC_CACHE = None


def run(hidden_states, gate_up_proj, down_proj, trace=False, **kw):
    """Run on the 8 NeuronCores; returns (output, BassKernelResults)."""
    global _NC_CACHE
    if _NC_CACHE is None:
        _NC_CACHE = build_nc()
    nc = _NC_CACHE

    hs = np.ascontiguousarray(np.asarray(hidden_states), dtype=np.float32)
    gup = np.ascontiguousarray(np.asarray(gate_up_proj), dtype=np.float32)
    dp = np.ascontiguousarray(np.asarray(down_proj), dtype=np.float32)
    assert hs.shape == (N_CORES * T, H), hs.shape
    assert gup.shape == (N_CORES, H, 2 * F), gup.shape
    assert dp.shape == (N_CORES, F, H), dp.shape

    in_maps = [
        {
            "hidden_states": np.ascontiguousarray(hs[i * T : (i + 1) * T].T),
            "gate_up_proj": np.ascontiguousarray(gup[i]),
            "down_proj": np.ascontiguousarray(dp[i]),
        }
        for i in range(N_CORES)
    ]
    res = run_bass_kernel_spmd(
        nc, in_maps, core_ids=list(range(N_CORES)), trace=trace, **kw
    )
    out = np.concatenate(
        [res.results[i]["out"] for i in range(N_CORES)], axis=0
    ).astype(np.float32)
    return out, res


def kernel(hidden_states, gate_up_proj, down_proj):
    out, _ = run(hidden_states, gate_up_proj, down_proj, trace=False)
    return out
